# revision 21
# baseline (speedup 1.0000x reference)
"""DigitCaps (CapsNet dynamic routing) Trainium2 kernel.

Strategy: shard N=4608 input capsules across 8 cores (576 each), keep the full
batch B=128 on every core.  u = x @ W is never materialized to HBM: it is
recomputed on the TensorEngine each routing sweep from SBUF-resident operands
(x packed as blockdiag stationary groups, W as blockdiag moving groups; 2
capsules per K=16 matmul group at base partitions {0,32,64,96}).  Routing
reductions over the output-capsule/vector axes run on VectorE/ScalarE straight
out of PSUM.  The per-iteration s-vector ([128,160] = 82KB) is combined across
cores with an AllReduce; all cores then compute identical squash/v and core 0's
output is returned.

Shapes (hardcoded): inputs [128,12,12,32,8], W [4608,8,160], biases [10,16].
"""
import sys

sys.path.insert(0, "/opt/trn_rl_repo")

import numpy as np

B = 128          # batch (full on every core)
NTOT = 4608      # input capsules
J = 8            # input capsule dim
C = 10           # output capsules
L = 16           # output capsule dim
CL = C * L       # 160
NCORES = 8
NL = NTOT // NCORES   # 576 capsules per core
NT = NL // 8          # 72 quad-blocks (8 capsules each: 4 bases x 2 packed)
EPS = 1e-7

_NC_CACHE = {}


def _build_nc(sim=False):
    import concourse.bass as bass  # noqa: F401
    import concourse.mybir as mybir
    import concourse.tile as tile
    import concourse.bacc as bacc

    f32 = mybir.dt.float32
    AX = mybir.AxisListType
    OP = mybir.AluOpType
    AF = mybir.ActivationFunctionType

    nc = bacc.Bacc("TRN2", target_bir_lowering=False, debug=False,
                   num_devices=1 if sim else NCORES)

    xs = nc.dram_tensor("xs", [128, NT * 128], f32, kind="ExternalInput")
    wm = nc.dram_tensor("wm", [128, NT * 320], f32, kind="ExternalInput")
    bias = nc.dram_tensor("bias", [128, CL], f32, kind="ExternalInput")
    out = nc.dram_tensor("out", [128, CL], f32, kind="ExternalOutput")
    RG = [list(range(NCORES))]

    with tile.TileContext(nc) as tc:
        with (
            tc.tile_pool(name="big", bufs=1) as big,
            tc.tile_pool(name="dram", bufs=1, space="DRAM") as dram,
            tc.tile_pool(name="wk", bufs=3) as wk,
            tc.tile_pool(name="sm", bufs=1) as sm,
        ):
            xs_sb = big.tile([128, NT * 128], f32, tag="xs_sb")
            wm_sb = big.tile([128, NT * 320], f32, tag="wm_sb")
            bias_sb = big.tile([128, CL], f32, tag="bias_sb")
            bstate = big.tile([128, NL * C], f32, tag="bstate")
            cstate = big.tile([128, NL * C], f32, tag="cstate")
            s_acc = big.tile([128, CL], f32, tag="s_acc")
            v_sb = big.tile([128, CL], f32, tag="v_sb")

            # chunked loads so the first matmuls can start early
            NCH = 8
            for ch in range(NCH):
                w0, w1 = ch * (NT // NCH) * 320, (ch + 1) * (NT // NCH) * 320
                nc.sync.dma_start(wm_sb[:, w0:w1], wm[:, w0:w1])
            for ch in range(NCH):
                x0, x1 = ch * (NT // NCH) * 128, (ch + 1) * (NT // NCH) * 128
                nc.sync.dma_start(xs_sb[:, x0:x1], xs[:, x0:x1])
            nc.sync.dma_start(bias_sb[:], bias[:])

            def quad_mms(pq, t, acc=False, first=False, last=False):
                """4 matmuls of quad-block t into psum tile pq [128,4,512].

                acc=False: each MM overwrites its own slot pq[:, vv, :320].
                acc=True:  each base keeps accumulating into its own slot
                (concurrent row-group MMs must never accumulate into shared
                addresses - that races in hardware).
                """
                for vv in range(4):
                    lhsT = xs_sb[32 * vv:32 * vv + 16, t * 128:(t + 1) * 128]
                    rhs = wm_sb[32 * vv:32 * vv + 16, t * 320:(t + 1) * 320]
                    tp = (96, 0) if vv == 3 else None
                    nc.tensor.matmul(
                        pq[:, vv, 0:320], lhsT, rhs,
                        start=(first if acc else True),
                        stop=(last if acc else True),
                        tile_position=tp, skip_group_check=acc)

            def all_reduce(src_sb, tag):
                cin = dram.tile([128, CL], f32, tag=f"cc_in_{tag}")
                cout = dram.tile([128, CL], f32, tag=f"cc_out_{tag}")
                nc.sync.dma_start(cin[:], src_sb[:])
                if not sim:
                    nc.gpsimd.collective_compute(
                        "AllReduce", OP.add, replica_groups=RG,
                        ins=[cin[:].opt()], outs=[cout[:].opt()])
                dst = sm.tile([128, CL], f32, tag="cc_sb")
                nc.sync.dma_start(dst[:], cin[:] if sim else cout[:])
                return dst

            def squash(s_red, scale, vout, tag):
                """vout = squash(scale*s_red + bias)."""
                ss = sm.tile([128, CL], f32, tag="ss")
                nc.scalar.activation(ss[:], s_red[:], AF.Copy, scale=float(scale))
                nc.vector.tensor_tensor(ss[:], ss[:], bias_sb[:], op=OP.add)
                sq = sm.tile([128, CL], f32, tag="sq")
                nc.vector.tensor_tensor(sq[:], ss[:], ss[:], op=OP.mult)
                n2 = sm.tile([128, C], f32, tag="n2")
                nc.vector.tensor_reduce(
                    n2[:], sq[:].rearrange("p (c l) -> p c l", l=L),
                    axis=AX.X, op=OP.add)
                rt = sm.tile([128, C], f32, tag="rt")
                nc.scalar.sqrt(rt[:], n2[:])
                d1 = sm.tile([128, C], f32, tag="d1")
                nc.vector.tensor_scalar_add(d1[:], n2[:], 1.0)
                d2 = sm.tile([128, C], f32, tag="d2")
                nc.vector.tensor_scalar_add(d2[:], rt[:], EPS)
                nc.vector.tensor_tensor(d1[:], d1[:], d2[:], op=OP.mult)
                rec = sm.tile([128, C], f32, tag="rec")
                nc.vector.reciprocal(rec[:], d1[:])
                nc.vector.tensor_tensor(rec[:], rec[:], n2[:], op=OP.mult)
                nc.vector.tensor_tensor(
                    vout[:].rearrange("p (c l) -> p c l", l=L),
                    ss[:].rearrange("p (c l) -> p c l", l=L),
                    rec[:].unsqueeze(2).to_broadcast([128, C, L]),
                    op=OP.mult)

            # ---------- iteration 0: c uniform -> s0 = sum(u)/C ----------
            with tc.tile_pool(name="ps_acc", bufs=1, space="PSUM") as ps_acc:
                pacc = ps_acc.tile([128, 4, 512], f32, tag="pacc")
                for t in range(NT):
                    quad_mms(pacc, t, acc=True, first=(t == 0),
                             last=(t == NT - 1))
                s0 = sm.tile([128, CL], f32, tag="s0")
                nc.vector.tensor_reduce(
                    s0[:],
                    pacc[:, :, 0:320].rearrange("p q (n k) -> p k q n", n=2),
                    axis=AX.XY, op=OP.add)
            s0f = all_reduce(s0, "r0")
            squash(s0f, 1.0 / C, v_sb, "r0")

            # ---------- iterations 1 and 2 ----------
            for it in (1, 2):
                with tc.tile_pool(name=f"ps_{it}z", bufs=2,
                                  space="PSUM") as psz:
                    for t in range(NT):
                        pq = psz.tile([128, 4, 512], f32, tag="pq")
                        quad_mms(pq, t)
                        u_ap = pq[:, :, 0:320].rearrange(
                            "p q (n c l) -> p q n c l", n=2, c=C, l=L)
                        zb = wk.tile([128, 1280], f32, tag="zb")
                        nc.vector.tensor_tensor(
                            zb[:].rearrange("p (q n c l) -> p q n c l",
                                            q=4, n=2, c=C, l=L),
                            u_ap,
                            v_sb[:].rearrange("p (c l) -> p c l", l=L)
                            .unsqueeze(1).unsqueeze(1)
                            .to_broadcast([128, 4, 2, C, L]),
                            op=OP.mult)
                        if it == 1:
                            nc.vector.tensor_reduce(
                                bstate[:, 80 * t:80 * (t + 1)],
                                zb[:].rearrange("p (a l) -> p a l", l=L),
                                axis=AX.X, op=OP.add)
                        else:
                            bt = wk.tile([128, 80], f32, tag="bt")
                            nc.vector.tensor_reduce(
                                bt[:],
                                zb[:].rearrange("p (a l) -> p a l", l=L),
                                axis=AX.X, op=OP.add)
                            nc.vector.tensor_tensor(
                                bstate[:, 80 * t:80 * (t + 1)],
                                bstate[:, 80 * t:80 * (t + 1)],
                                bt[:], op=OP.add)

                # softmax over c (groups of 10 in bstate cols)
                nc.scalar.activation(cstate[:], bstate[:], AF.Exp)
                den = sm.tile([128, NL], f32, tag="den")
                nc.vector.tensor_reduce(
                    den[:], cstate[:].rearrange("p (n c) -> p n c", c=C),
                    axis=AX.X, op=OP.add)
                rec = sm.tile([128, NL], f32, tag="recn")
                nc.vector.reciprocal(rec[:], den[:])
                nc.vector.tensor_tensor(
                    cstate[:].rearrange("p (n c) -> p n c", c=C),
                    cstate[:].rearrange("p (n c) -> p n c", c=C),
                    rec[:].unsqueeze(2).to_broadcast([128, NL, C]),
                    op=OP.mult)

                # s = sum_n c*u  (second einsum sweep)
                nc.any.memzero(s_acc)
                with tc.tile_pool(name=f"ps_{it}y", bufs=2,
                                  space="PSUM") as psy:
                    for t in range(NT):
                        pq = psy.tile([128, 4, 512], f32, tag="pqy")
                        quad_mms(pq, t)
                        yb = wk.tile([128, 1280], f32, tag="zb")
                        nc.vector.tensor_tensor(
                            yb[:].rearrange("p (c l q n) -> p c l q n",
                                            c=C, l=L, q=4, n=2),
                            pq[:, :, 0:320].rearrange(
                                "p q (n c l) -> p c l q n", n=2, c=C, l=L),
                            cstate[:, 80 * t:80 * (t + 1)].rearrange(
                                "p (q n c) -> p c q n", q=4, n=2)
                            .unsqueeze(2).to_broadcast([128, C, L, 4, 2]),
                            op=OP.mult)
                        st = wk.tile([128, CL], f32, tag="st")
                        nc.vector.tensor_reduce(
                            st[:],
                            yb[:].rearrange("p (k a) -> p k a", a=8),
                            axis=AX.X, op=OP.add)
                        nc.vector.tensor_tensor(
                            s_acc[:], s_acc[:], st[:], op=OP.add)
                sf = all_reduce(s_acc, f"r{it}")
                squash(sf, 1.0, v_sb, f"r{it}")

            nc.sync.dma_start(out[:], v_sb[:])

    nc.compile()
    return nc


def _build_nc_v2(sim=False, dbg=False):
    """No-u design: routing contractions on TensorE in n-on-partitions layout.

    Per iteration: bupd = x*(W@v) via Wv-matmuls + delta-matmul partition-group
    reduces; softmax in [(n),(c,b)] layout; s via y=c*x as matmul stationary
    against W. PSUM accumulation replaces all big DVE reductions.
    """
    import concourse.bass as bass  # noqa: F401
    import concourse.mybir as mybir
    import concourse.tile as tile
    import concourse.bacc as bacc
    from concourse.masks import make_identity

    f32 = mybir.dt.float32
    bf16 = mybir.dt.bfloat16
    AX = mybir.AxisListType
    OP = mybir.AluOpType
    AF = mybir.ActivationFunctionType

    nc = bacc.Bacc("TRN2", target_bir_lowering=False, debug=False,
                   num_devices=1 if sim else NCORES)

    xt_d = nc.dram_tensor("xt", [128, 5, 8, 128], bf16, kind="ExternalInput")
    xnj_d = nc.dram_tensor("xnj", [128, 5, 8, 128], bf16, kind="ExternalInput")
    wt4_d = nc.dram_tensor("wt4", [128, 5, 8, 160], bf16, kind="ExternalInput")
    wt6_d = nc.dram_tensor("wt6", [128, 3, 5, 8, 128], bf16, kind="ExternalInput")
    delta_d = nc.dram_tensor("delta", [128, 32], bf16, kind="ExternalInput")
    bias = nc.dram_tensor("bias", [128, CL], f32, kind="ExternalInput")
    out = nc.dram_tensor("out", [128, CL], f32, kind="ExternalOutput")
    if dbg:
        s0_dbg = nc.dram_tensor("s0_dbg", [128, CL], f32, kind="ExternalOutput")
        v0_dbg = nc.dram_tensor("v0_dbg", [128, CL], f32, kind="ExternalOutput")
        b1_dbg = nc.dram_tensor("b1_dbg", [128, 5, 10, 128], f32, kind="ExternalOutput")
        c1_dbg = nc.dram_tensor("c1_dbg", [128, 5, 10, 128], f32, kind="ExternalOutput")
        s1_dbg = nc.dram_tensor("s1_dbg", [128, CL], f32, kind="ExternalOutput")
    RG = [list(range(NCORES))]

    with tile.TileContext(nc) as tc:
        with (
            tc.tile_pool(name="big", bufs=1) as big,
            tc.tile_pool(name="dram", bufs=1, space="DRAM") as dram,
            tc.tile_pool(name="wk", bufs=3) as wk,
            tc.tile_pool(name="sm", bufs=1) as sm,
        ):
            xt = big.tile([128, 5, 8, 128], bf16, tag="xt")
            xnj = big.tile([128, 5, 8, 128], bf16, tag="xnj")
            wt4 = big.tile([128, 5, 8, 160], bf16, tag="wt4")
            wt6 = big.tile([128, 3, 5, 8, 128], bf16, tag="wt6")
            delta_sb = big.tile([128, 32], bf16, tag="delta")
            bias_sb = big.tile([128, CL], f32, tag="bias_sb")
            ident = big.tile([128, 128], f32, tag="ident")
            bstateT = big.tile([128, 5, 10, 128], f32, tag="bstateT")
            cstateT = big.tile([128, 5, 10, 128], bf16, tag="cstateT")
            den_t = big.tile([128, 5, 128], f32, tag="den_t")
            rec_t = big.tile([128, 5, 128], f32, tag="rec_t")
            vT = big.tile([128, 3, 128], bf16, tag="vT")
            v_sb = big.tile([128, CL], f32, tag="v_sb")

            make_identity(nc, ident[:])
            for G in range(5):
                nc.sync.dma_start(xt[:, G], xt_d[:, G])
                nc.sync.dma_start(xnj[:, G], xnj_d[:, G])
                nc.sync.dma_start(wt4[:, G], wt4_d[:, G])
            for cyc in range(3):
                nc.sync.dma_start(wt6[:, cyc], wt6_d[:, cyc])
            nc.sync.dma_start(delta_sb[:], delta_d[:])
            nc.sync.dma_start(bias_sb[:], bias[:])

            def all_reduce(src_sb, tag):
                cin = dram.tile([128, CL], f32, tag=f"cc_in_{tag}")
                cout = dram.tile([128, CL], f32, tag=f"cc_out_{tag}")
                nc.sync.dma_start(cin[:], src_sb[:])
                if not sim:
                    nc.gpsimd.collective_compute(
                        "AllReduce", OP.add, replica_groups=RG,
                        ins=[cin[:].opt()], outs=[cout[:].opt()])
                dst = sm.tile([128, CL], f32, tag="cc_sb")
                nc.sync.dma_start(dst[:], cin[:] if sim else cout[:])
                return dst

            def squash(s_red, scale, vout):
                ss = sm.tile([128, CL], f32, tag="ss")
                nc.scalar.activation(ss[:], s_red[:], AF.Copy, scale=float(scale))
                nc.vector.tensor_tensor(ss[:], ss[:], bias_sb[:], op=OP.add)
                sq = sm.tile([128, CL], f32, tag="sq")
                nc.vector.tensor_tensor(sq[:], ss[:], ss[:], op=OP.mult)
                n2 = sm.tile([128, C], f32, tag="n2")
                nc.vector.tensor_reduce(
                    n2[:], sq[:].rearrange("p (c l) -> p c l", l=L),
                    axis=AX.X, op=OP.add)
                rt = sm.tile([128, C], f32, tag="rt")
                nc.scalar.sqrt(rt[:], n2[:])
                d1 = sm.tile([128, C], f32, tag="d1")
                nc.vector.tensor_scalar_add(d1[:], n2[:], 1.0)
                d2 = sm.tile([128, C], f32, tag="d2")
                nc.vector.tensor_scalar_add(d2[:], rt[:], EPS)
                nc.vector.tensor_tensor(d1[:], d1[:], d2[:], op=OP.mult)
                rec = sm.tile([128, C], f32, tag="rec")
                nc.vector.reciprocal(rec[:], d1[:])
                nc.vector.tensor_tensor(rec[:], rec[:], n2[:], op=OP.mult)
                nc.vector.tensor_tensor(
                    vout[:].rearrange("p (c l) -> p c l", l=L),
                    ss[:].rearrange("p (c l) -> p c l", l=L),
                    rec[:].unsqueeze(2).to_broadcast([128, C, L]),
                    op=OP.mult)

            def build_vT(vsrc):
                with tc.tile_pool(name="pst", bufs=2, space="PSUM") as pst:
                    for c in range(10):
                        slot, cyc = c % 4, c // 4
                        tp = pst.tile([16, 128], f32, tag="tp")
                        nc.tensor.transpose(
                            tp[:], vsrc[:, 16 * c:16 * c + 16], ident[:])
                        nc.scalar.copy(
                            vT[32 * slot:32 * slot + 16, cyc, :], tp[:])

            def s_from_psum(sp):
                s_sb = sm.tile([128, CL], f32, tag="s_sb")
                nc.vector.tensor_copy(
                    s_sb[:].rearrange("p (c l) -> p c l", l=L), sp[:])
                return s_sb

            # ---------- iteration 0: s0 = (1/C) sum_{n,j} x W ----------
            with tc.tile_pool(name="ps0", bufs=1, space="PSUM") as ps0:
                sp = ps0.tile([128, 10, 16], f32, tag="sp0")
                for c in range(10):
                    for G in range(5):
                        for j in range(8):
                            nc.tensor.matmul(
                                sp[:, c, :], xt[:, G, j, :],
                                wt4[:, G, j, 16 * c:16 * c + 16],
                                start=(G == 0 and j == 0),
                                stop=(G == 4 and j == 7),
                                skip_group_check=True)
                s0 = s_from_psum(sp)
            if dbg:
                nc.sync.dma_start(s0_dbg[:], s0[:])
            s0f = all_reduce(s0, "r0")
            squash(s0f, 1.0 / C, v_sb)
            if dbg:
                nc.sync.dma_start(v0_dbg[:], v_sb[:])
            build_vT(v_sb)

            # ---------- iterations 1 and 2 ----------
            for it in (1, 2):
                with (tc.tile_pool(name=f"wv{it}", bufs=2, space="PSUM") as wvp,
                      tc.tile_pool(name=f"bu{it}", bufs=1, space="PSUM") as bup):
                    for G in range(5):
                        bq = bup.tile([128, 10, 128], f32, tag="bq")
                        for c in range(10):
                            slot, cyc = c % 4, c // 4
                            wq = wvp.tile([128, 8, 128], f32, tag="wq")
                            for h in range(8):
                                nc.tensor.matmul(
                                    wq[:, h, :],
                                    wt6[32 * slot:32 * slot + 16, cyc, G, h, :],
                                    vT[32 * slot:32 * slot + 16, cyc, :],
                                    start=True, stop=True,
                                    tile_position=(96, 0) if slot == 3 else None)
                            wvs = wk.tile([128, 8, 128], bf16, tag="wvs")
                            nc.scalar.copy(wvs[:], wq[:])
                            zt = wk.tile([128, 8, 128], bf16, tag="zt")
                            nc.vector.tensor_tensor(
                                zt[:], wvs[:], xnj[:, G], op=OP.mult)
                            for s4 in range(4):
                                for ii, h in enumerate((s4, s4 + 4)):
                                    nc.tensor.matmul(
                                        bq[32 * s4:32 * s4 + 32, c, :],
                                        delta_sb[:], zt[:, h, :],
                                        start=(ii == 0), stop=(ii == 1),
                                        tile_position=(0, 32 * s4),
                                        skip_group_check=True)
                        if it == 1:
                            nc.scalar.copy(bstateT[:, G], bq[:])
                        else:
                            nc.vector.tensor_tensor(
                                bstateT[:, G], bstateT[:, G], bq[:], op=OP.add)

                if dbg and it == 1:
                    nc.sync.dma_start(b1_dbg[:], bstateT[:])
                # softmax over c in [(n), (c, b)] layout
                nc.scalar.activation(cstateT[:], bstateT[:], AF.Exp)
                for G in range(5):
                    nc.vector.tensor_reduce(
                        den_t[:, G, :],
                        cstateT[:, G].rearrange("p c b -> p b c"),
                        axis=AX.X, op=OP.add)
                nc.vector.reciprocal(rec_t[:], den_t[:])
                for G in range(5):
                    nc.vector.tensor_tensor(
                        cstateT[:, G], cstateT[:, G],
                        rec_t[:, G, :].unsqueeze(1).to_broadcast([128, 10, 128]),
                        op=OP.mult)

                if dbg and it == 1:
                    cdbg = big.tile([128, 5, 10, 128], f32, tag="cdbg")
                    nc.vector.tensor_copy(cdbg[:], cstateT[:])
                    nc.sync.dma_start(c1_dbg[:], cdbg[:])
                # s = sum_{n,j} (c*x) W  via y-stationary matmuls
                # NOTE: accumulation chains must be sequential per PSUM
                # region - interleaved chains sharing a bank corrupt results.
                with tc.tile_pool(name=f"sps{it}", bufs=1, space="PSUM") as sps:
                    sp = sps.tile([128, 10, 16], f32, tag="spi")
                    for c in range(10):
                        for G in range(5):
                            y = wk.tile([128, 8, 128], bf16, tag="y")
                            nc.vector.tensor_tensor(
                                y[:],
                                cstateT[:, G, c, :].unsqueeze(1)
                                .to_broadcast([128, 8, 128]),
                                xt[:, G], op=OP.mult)
                            for j in range(8):
                                nc.tensor.matmul(
                                    sp[:, c, :], y[:, j, :],
                                    wt4[:, G, j, 16 * c:16 * c + 16],
                                    start=(G == 0 and j == 0),
                                    stop=(G == 4 and j == 7),
                                    skip_group_check=True)
                    s_it = s_from_psum(sp)
                if dbg and it == 1:
                    nc.sync.dma_start(s1_dbg[:], s_it[:])
                sf = all_reduce(s_it, f"r{it}")
                squash(sf, 1.0, v_sb)
                if it == 1:
                    build_vT(v_sb)

            nc.sync.dma_start(out[:], v_sb[:])

    nc.compile()
    return nc


def _build_nc_v3(sim=False):
    """v3: G-pipelined iterations. Same inputs/host-prep as v2.

    Differences vs v2:
    - iter0 uses 40 matmuls with 160-wide rhs (was 400 x 16-wide).
    - iterations fuse bupd -> softmax -> s into ONE loop over G so the
      Tile scheduler can overlap TensorE/Scalar/Vector across G-blocks
      (per-G chains are independent; s accumulates in per-c PSUM chains
      ordered G0..G4).
    - softmax normalize is folded into x: xhat = x * (1/den), y = e * xhat
      (skips the cstate normalize pass; everything stays bf16 2x).
    - half the wq->SBUF copies go to VectorE to unblock ScalarE.
    """
    import concourse.bass as bass  # noqa: F401
    import concourse.mybir as mybir
    import concourse.tile as tile
    import concourse.bacc as bacc
    from concourse.masks import make_identity

    f32 = mybir.dt.float32
    bf16 = mybir.dt.bfloat16
    AX = mybir.AxisListType
    OP = mybir.AluOpType
    AF = mybir.ActivationFunctionType

    nc = bacc.Bacc("TRN2", target_bir_lowering=False, debug=False,
                   num_devices=1 if sim else NCORES)

    xt_d = nc.dram_tensor("xt", [128, 5, 8, 128], bf16, kind="ExternalInput")
    xnj_d = nc.dram_tensor("xnj", [128, 5, 8, 128], bf16, kind="ExternalInput")
    wt4_d = nc.dram_tensor("wt4", [128, 5, 8, 160], bf16, kind="ExternalInput")
    wt6_d = nc.dram_tensor("wt6", [128, 3, 5, 8, 128], bf16, kind="ExternalInput")
    delta_d = nc.dram_tensor("delta", [128, 32], bf16, kind="ExternalInput")
    bias = nc.dram_tensor("bias", [128, CL], f32, kind="ExternalInput")
    out = nc.dram_tensor("out", [128, CL], f32, kind="ExternalOutput")
    RG = [list(range(NCORES))]

    with tile.TileContext(nc) as tc:
        with (
            tc.tile_pool(name="big", bufs=1) as big,
            tc.tile_pool(name="dram", bufs=1, space="DRAM") as dram,
            tc.tile_pool(name="wk", bufs=4) as wk,
            tc.tile_pool(name="sm", bufs=1) as sm,
        ):
            xt = big.tile([128, 5, 8, 128], bf16, tag="xt")
            xnj = big.tile([128, 5, 8, 128], bf16, tag="xnj")
            wt4 = big.tile([128, 5, 8, 160], bf16, tag="wt4")
            wt6 = big.tile([128, 3, 5, 8, 128], bf16, tag="wt6")
            delta_sb = big.tile([128, 32], bf16, tag="delta")
            bias_sb = big.tile([128, CL], f32, tag="bias_sb")
            ident = big.tile([128, 128], f32, tag="ident")
            bstateT = big.tile([128, 5, 10, 128], f32, tag="bstateT")
            cstateT = big.tile([128, 5, 10, 128], bf16, tag="cstateT")
            vT = big.tile([128, 3, 128], bf16, tag="vT")
            v_sb = big.tile([128, CL], f32, tag="v_sb")

            make_identity(nc, ident[:])
            # Warm-up barrier: a tiny AllReduce issued first so the 8 cores
            # rendezvous while input DMA + iter0 matmuls run.  Without it the
            # first real AllReduce absorbs all the launch skew (~25us).
            warm_in = dram.tile([128, 4], f32, tag="warm_in")
            warm_out = dram.tile([128, 4], f32, tag="warm_out")
            warm_sb = sm.tile([128, 4], f32, tag="warm_sb")
            nc.any.memzero(warm_sb)
            nc.sync.dma_start(warm_in[:], warm_sb[:])
            if not sim:
                nc.gpsimd.collective_compute(
                    "AllReduce", OP.add, replica_groups=RG,
                    ins=[warm_in[:].opt()], outs=[warm_out[:].opt()])
            # iter0 needs only xt+wt4: load those first so matmuls start
            # early; xnj/wt6/delta (iteration-1 operands) stream in behind,
            # overlapped with iter0 compute + the first collective.
            for G in range(5):
                nc.sync.dma_start(xt[:, G], xt_d[:, G])
                nc.sync.dma_start(wt4[:, G], wt4_d[:, G])
            nc.sync.dma_start(bias_sb[:], bias[:])
            for G in range(5):
                nc.sync.dma_start(xnj[:, G], xnj_d[:, G])
            nc.sync.dma_start(delta_sb[:], delta_d[:])
            for cyc in range(3):
                nc.sync.dma_start(wt6[:, cyc], wt6_d[:, cyc])

            def all_reduce(src_sb, tag):
                cin = dram.tile([128, CL], f32, tag=f"cc_in_{tag}")
                cout = dram.tile([128, CL], f32, tag=f"cc_out_{tag}")
                nc.sync.dma_start(cin[:], src_sb[:])
                if not sim:
                    nc.gpsimd.collective_compute(
                        "AllReduce", OP.add, replica_groups=RG,
                        ins=[cin[:].opt()], outs=[cout[:].opt()])
                dst = sm.tile([128, CL], f32, tag="cc_sb")
                nc.sync.dma_start(dst[:], cin[:] if sim else cout[:])
                return dst

            def squash(s_red, scale, vout):
                ss = sm.tile([128, CL], f32, tag="ss")
                nc.scalar.activation(ss[:], s_red[:], AF.Copy, scale=float(scale))
                nc.vector.tensor_tensor(ss[:], ss[:], bias_sb[:], op=OP.add)
                sq = sm.tile([128, CL], f32, tag="sq")
                nc.vector.tensor_tensor(sq[:], ss[:], ss[:], op=OP.mult)
                n2 = sm.tile([128, C], f32, tag="n2")
                nc.vector.tensor_reduce(
                    n2[:], sq[:].rearrange("p (c l) -> p c l", l=L),
                    axis=AX.X, op=OP.add)
                rt = sm.tile([128, C], f32, tag="rt")
                nc.scalar.sqrt(rt[:], n2[:])
                d1 = sm.tile([128, C], f32, tag="d1")
                nc.vector.tensor_scalar_add(d1[:], n2[:], 1.0)
                d2 = sm.tile([128, C], f32, tag="d2")
                nc.vector.tensor_scalar_add(d2[:], rt[:], EPS)
                nc.vector.tensor_tensor(d1[:], d1[:], d2[:], op=OP.mult)
                rec = sm.tile([128, C], f32, tag="rec")
                nc.vector.reciprocal(rec[:], d1[:])
                nc.vector.tensor_tensor(rec[:], rec[:], n2[:], op=OP.mult)
                nc.vector.tensor_tensor(
                    vout[:].rearrange("p (c l) -> p c l", l=L),
                    ss[:].rearrange("p (c l) -> p c l", l=L),
                    rec[:].unsqueeze(2).to_broadcast([128, C, L]),
                    op=OP.mult)

            def build_vT(vsrc):
                with tc.tile_pool(name="pst", bufs=2, space="PSUM") as pst:
                    for c in range(10):
                        slot, cyc = c % 4, c // 4
                        tp = pst.tile([16, 128], f32, tag="tp")
                        nc.tensor.transpose(
                            tp[:], vsrc[:, 16 * c:16 * c + 16], ident[:])
                        nc.scalar.copy(
                            vT[32 * slot:32 * slot + 16, cyc, :], tp[:])

            # ---------- iteration 0: s0 = (1/C) sum_{n,j} x W ----------
            with tc.tile_pool(name="ps0", bufs=1, space="PSUM") as ps0:
                sp = ps0.tile([128, 10, 16], f32, tag="sp0")
                for G in range(5):
                    for j in range(8):
                        nc.tensor.matmul(
                            sp[:].rearrange("p c l -> p (c l)"),
                            xt[:, G, j, :], wt4[:, G, j, :],
                            start=(G == 0 and j == 0),
                            stop=(G == 4 and j == 7))
                s0 = sm.tile([128, CL], f32, tag="s_sb0")
                nc.vector.tensor_copy(
                    s0[:].rearrange("p (c l) -> p c l", l=L), sp[:])
            s0f = all_reduce(s0, "r0")
            squash(s0f, 1.0 / C, v_sb)
            build_vT(v_sb)

            # ---------- iterations 1 and 2 (G-pipelined) ----------
            for it in (1, 2):
                with (
                    tc.tile_pool(name=f"wv{it}", bufs=2, space="PSUM") as wvp,
                    tc.tile_pool(name=f"bu{it}", bufs=1, space="PSUM") as bup,
                    tc.tile_pool(name=f"sp{it}", bufs=1, space="PSUM") as spp,
                ):
                    s_acc = sm.tile([128, CL], f32, tag=f"sacc{it}")
                    for G in range(5):
                        # --- b-update for this G ---
                        bq = bup.tile([128, 10, 128], f32, tag="bq")
                        for c in range(10):
                            slot, cyc = c % 4, c // 4
                            wq = wvp.tile([128, 8, 128], f32, tag="wq")
                            for h in range(8):
                                nc.tensor.matmul(
                                    wq[:, h, :],
                                    wt6[32 * slot:32 * slot + 16, cyc, G, h, :],
                                    vT[32 * slot:32 * slot + 16, cyc, :],
                                    start=True, stop=True,
                                    tile_position=(96, 0) if slot == 3
                                    else None)
                            wvs = wk.tile([128, 8, 128], bf16, tag="wvs")
                            nc.scalar.copy(wvs[:], wq[:])
                            zt = wk.tile([128, 8, 128], bf16, tag="zt")
                            nc.vector.tensor_tensor(
                                zt[:], wvs[:], xnj[:, G], op=OP.mult)
                            for s4 in range(4):
                                for ii, h in enumerate((s4, s4 + 4)):
                                    nc.tensor.matmul(
                                        bq[32 * s4:32 * s4 + 32, c, :],
                                        delta_sb[:], zt[:, h, :],
                                        start=(ii == 0), stop=(ii == 1),
                                        tile_position=(0, 32 * s4),
                                        skip_group_check=True)
                        if it == 1:
                            nc.scalar.copy(bstateT[:, G], bq[:])
                        else:
                            nc.vector.tensor_tensor(
                                bstateT[:, G], bstateT[:, G], bq[:], op=OP.add)

                        # --- softmax for this G (normalize folded into x) ---
                        nc.scalar.activation(
                            cstateT[:, G], bstateT[:, G], AF.Exp)
                        den = sm.tile([128, 128], f32, tag=f"den{it}_{G}")
                        nc.vector.tensor_reduce(
                            den[:],
                            cstateT[:, G].rearrange("p c b -> p b c"),
                            axis=AX.X, op=OP.add)
                        recb = sm.tile([128, 128], bf16, tag=f"recb{it}_{G}")
                        with nc.allow_low_precision(
                                reason="softmax 1/den in bf16; rel tol 2e-2"):
                            nc.vector.reciprocal(recb[:], den[:])
                        xh = wk.tile([128, 8, 128], bf16, tag="xh")
                        nc.vector.tensor_tensor(
                            xh[:], xt[:, G],
                            recb[:].unsqueeze(1).to_broadcast([128, 8, 128]),
                            op=OP.mult)

                        # --- s contribution for this G (per-c chains close
                        # within G; cross-G accumulation in SBUF to avoid
                        # interleaved chains sharing a PSUM bank) ---
                        sp = spp.tile([128, 10, 16], f32, tag="spi")
                        for c in range(10):
                            y = wk.tile([128, 8, 128], bf16, tag="y")
                            yeng = nc.vector if c < 8 else nc.gpsimd
                            yeng.tensor_tensor(
                                y[:],
                                cstateT[:, G, c, :].unsqueeze(1)
                                .to_broadcast([128, 8, 128]),
                                xh[:], op=OP.mult)
                            for j in range(8):
                                nc.tensor.matmul(
                                    sp[:, c, :], y[:, j, :],
                                    wt4[:, G, j, 16 * c:16 * c + 16],
                                    start=(j == 0), stop=(j == 7),
                                    skip_group_check=True)
                        if G == 0:
                            nc.vector.tensor_copy(
                                s_acc[:].rearrange("p (c l) -> p c l", l=L),
                                sp[:])
                        else:
                            nc.vector.tensor_tensor(
                                s_acc[:].rearrange("p (c l) -> p c l", l=L),
                                s_acc[:].rearrange("p (c l) -> p c l", l=L),
                                sp[:], op=OP.add)
                    s_it = s_acc
                sf = all_reduce(s_it, f"r{it}")
                squash(sf, 1.0, v_sb)
                if it == 1:
                    build_vT(v_sb)

            nc.sync.dma_start(out[:], v_sb[:])

    nc.compile()
    return nc


def _prep_core_v2(x_shard, W_shard):
    """Host layouts for v2. x_shard [128,576,8] f32, W_shard [576,8,160]."""
    import ml_dtypes
    bf = ml_dtypes.bfloat16
    xp = np.zeros((128, 640, 8), np.float32)
    xp[:, :NL] = x_shard
    Wp = np.zeros((640, 8, 10, 16), np.float32)
    Wp[:NL] = W_shard.reshape(NL, 8, 10, 16)

    # xt[nn, G, j, b] = xp[b, 128G+nn, j]
    xt = xp.reshape(128, 5, 128, 8).transpose(2, 1, 3, 0).copy()
    # xnj[4*ns+jj, G, 4*jh+s, b] = xp[b, 128G+32s+ns, 4jh+jj]
    t = xp.reshape(128, 5, 4, 32, 2, 4)           # [b, G, s, ns, jh, jj]
    xnj = (t.transpose(3, 5, 1, 4, 2, 0)          # [ns, jj, G, jh, s, b]
           .reshape(128, 5, 8, 128))
    # wt4[nn, G, j, (c l)] = Wp[128G+nn, j, c, l]
    wt4 = (Wp.reshape(5, 128, 8, 160).transpose(1, 0, 2, 3)).copy()
    # wt6[32*(c%4)+l, c//4, G, 4*jh+s, 4*ns+jj] = Wp[128G+32s+ns, 4jh+jj, c, l]
    wt6 = np.zeros((128, 3, 5, 8, 128), np.float32)
    t6 = Wp.reshape(5, 4, 32, 2, 4, 10, 16)       # [G, s, ns, jh, jj, c, l]
    for c in range(10):
        blk = t6[:, :, :, :, :, c, :]             # [G, s, ns, jh, jj, l]
        arr = (blk.transpose(5, 0, 3, 1, 2, 4)    # [l, G, jh, s, ns, jj]
               .reshape(16, 5, 8, 128))
        wt6[32 * (c % 4):32 * (c % 4) + 16, c // 4] = arr
    delta = np.zeros((128, 32), np.float32)
    for ns in range(32):
        delta[4 * ns:4 * ns + 4, ns] = 1.0
    return (xt.astype(bf), xnj.astype(bf), wt4.astype(bf), wt6.astype(bf),
            delta.astype(bf))


def _prep_core(x_shard, W_shard):
    """x_shard [B,576,8] f32, W_shard [576,8,160] f32 -> (xs, wm) operands."""
    xs = np.zeros((128, NT, 128), np.float32)
    wmv = np.zeros((128, NT, 320), np.float32)
    xr = x_shard.transpose(1, 2, 0).reshape(NT, 4, 2, J, B)
    wr = W_shard.reshape(NT, 4, 2, J, CL)
    for vv in range(4):
        for ns in range(2):
            rows = slice(32 * vv + 8 * ns, 32 * vv + 8 * ns + 8)
            xs[rows] = xr[:, vv, ns].transpose(1, 0, 2)
            wmv[rows, :, CL * ns:CL * (ns + 1)] = wr[:, vv, ns].transpose(1, 0, 2)
    return xs.reshape(128, NT * 128), wmv.reshape(128, NT * 320)


def prepare(inputs):
    """Build (nc, in_maps) for the current kernel version."""
    x = np.ascontiguousarray(inputs["inputs"], dtype=np.float32)
    W = np.ascontiguousarray(inputs["W"], dtype=np.float32)
    biases = np.ascontiguousarray(inputs["biases"], dtype=np.float32)
    x = x.reshape(B, NTOT, J)
    bias_rep = np.broadcast_to(biases.reshape(1, CL), (128, CL)).copy()

    import os
    ver = os.environ.get("KERNEL_V", "3")
    in_maps = []
    for i in range(NCORES):
        sl = slice(i * NL, (i + 1) * NL)
        if ver in ("2", "3"):
            xt, xnj, wt4, wt6, delta = _prep_core_v2(x[:, sl], W[sl])
            in_maps.append({"xt": xt, "xnj": xnj, "wt4": wt4, "wt6": wt6,
                            "delta": delta, "bias": bias_rep})
        else:
            xs, wmv = _prep_core(x[:, sl], W[sl])
            in_maps.append({"xs": xs, "wm": wmv, "bias": bias_rep})

    key = f"nc{ver}"
    if key not in _NC_CACHE:
        builder = {"1": _build_nc, "2": _build_nc_v2, "3": _build_nc_v3}[ver]
        _NC_CACHE[key] = builder()
    return _NC_CACHE[key], in_maps


def kernel(**inputs):
    from concourse.bass_utils import run_bass_kernel_spmd

    nc, in_maps = prepare(inputs)
    res = run_bass_kernel_spmd(nc, in_maps, core_ids=list(range(NCORES)))
    return res.results[0]["out"].reshape(B, C, L).astype(np.float32)



# revision 22
# speedup vs baseline: 1.0609x; 1.0609x over previous
"""DigitCaps (CapsNet dynamic routing) Trainium2 kernel.

Strategy: shard N=4608 input capsules across 8 cores (576 each), keep the full
batch B=128 on every core.  u = x @ W is never materialized to HBM: it is
recomputed on the TensorEngine each routing sweep from SBUF-resident operands
(x packed as blockdiag stationary groups, W as blockdiag moving groups; 2
capsules per K=16 matmul group at base partitions {0,32,64,96}).  Routing
reductions over the output-capsule/vector axes run on VectorE/ScalarE straight
out of PSUM.  The per-iteration s-vector ([128,160] = 82KB) is combined across
cores with an AllReduce; all cores then compute identical squash/v and core 0's
output is returned.

Shapes (hardcoded): inputs [128,12,12,32,8], W [4608,8,160], biases [10,16].
"""
import sys

sys.path.insert(0, "/opt/trn_rl_repo")

import numpy as np

B = 128          # batch (full on every core)
NTOT = 4608      # input capsules
J = 8            # input capsule dim
C = 10           # output capsules
L = 16           # output capsule dim
CL = C * L       # 160
NCORES = 8
NL = NTOT // NCORES   # 576 capsules per core
NT = NL // 8          # 72 quad-blocks (8 capsules each: 4 bases x 2 packed)
EPS = 1e-7

_NC_CACHE = {}


def _build_nc(sim=False):
    import concourse.bass as bass  # noqa: F401
    import concourse.mybir as mybir
    import concourse.tile as tile
    import concourse.bacc as bacc

    f32 = mybir.dt.float32
    AX = mybir.AxisListType
    OP = mybir.AluOpType
    AF = mybir.ActivationFunctionType

    nc = bacc.Bacc("TRN2", target_bir_lowering=False, debug=False,
                   num_devices=1 if sim else NCORES)

    xs = nc.dram_tensor("xs", [128, NT * 128], f32, kind="ExternalInput")
    wm = nc.dram_tensor("wm", [128, NT * 320], f32, kind="ExternalInput")
    bias = nc.dram_tensor("bias", [128, CL], f32, kind="ExternalInput")
    out = nc.dram_tensor("out", [128, CL], f32, kind="ExternalOutput")
    RG = [list(range(NCORES))]

    with tile.TileContext(nc) as tc:
        with (
            tc.tile_pool(name="big", bufs=1) as big,
            tc.tile_pool(name="dram", bufs=1, space="DRAM") as dram,
            tc.tile_pool(name="wk", bufs=3) as wk,
            tc.tile_pool(name="sm", bufs=1) as sm,
        ):
            xs_sb = big.tile([128, NT * 128], f32, tag="xs_sb")
            wm_sb = big.tile([128, NT * 320], f32, tag="wm_sb")
            bias_sb = big.tile([128, CL], f32, tag="bias_sb")
            bstate = big.tile([128, NL * C], f32, tag="bstate")
            cstate = big.tile([128, NL * C], f32, tag="cstate")
            s_acc = big.tile([128, CL], f32, tag="s_acc")
            v_sb = big.tile([128, CL], f32, tag="v_sb")

            # chunked loads so the first matmuls can start early
            NCH = 8
            for ch in range(NCH):
                w0, w1 = ch * (NT // NCH) * 320, (ch + 1) * (NT // NCH) * 320
                nc.sync.dma_start(wm_sb[:, w0:w1], wm[:, w0:w1])
            for ch in range(NCH):
                x0, x1 = ch * (NT // NCH) * 128, (ch + 1) * (NT // NCH) * 128
                nc.sync.dma_start(xs_sb[:, x0:x1], xs[:, x0:x1])
            nc.sync.dma_start(bias_sb[:], bias[:])

            def quad_mms(pq, t, acc=False, first=False, last=False):
                """4 matmuls of quad-block t into psum tile pq [128,4,512].

                acc=False: each MM overwrites its own slot pq[:, vv, :320].
                acc=True:  each base keeps accumulating into its own slot
                (concurrent row-group MMs must never accumulate into shared
                addresses - that races in hardware).
                """
                for vv in range(4):
                    lhsT = xs_sb[32 * vv:32 * vv + 16, t * 128:(t + 1) * 128]
                    rhs = wm_sb[32 * vv:32 * vv + 16, t * 320:(t + 1) * 320]
                    tp = (96, 0) if vv == 3 else None
                    nc.tensor.matmul(
                        pq[:, vv, 0:320], lhsT, rhs,
                        start=(first if acc else True),
                        stop=(last if acc else True),
                        tile_position=tp, skip_group_check=acc)

            def all_reduce(src_sb, tag):
                cin = dram.tile([128, CL], f32, tag=f"cc_in_{tag}")
                cout = dram.tile([128, CL], f32, tag=f"cc_out_{tag}")
                nc.sync.dma_start(cin[:], src_sb[:])
                if not sim:
                    nc.gpsimd.collective_compute(
                        "AllReduce", OP.add, replica_groups=RG,
                        ins=[cin[:].opt()], outs=[cout[:].opt()])
                dst = sm.tile([128, CL], f32, tag="cc_sb")
                nc.sync.dma_start(dst[:], cin[:] if sim else cout[:])
                return dst

            def squash(s_red, scale, vout, tag):
                """vout = squash(scale*s_red + bias)."""
                ss = sm.tile([128, CL], f32, tag="ss")
                nc.scalar.activation(ss[:], s_red[:], AF.Copy, scale=float(scale))
                nc.vector.tensor_tensor(ss[:], ss[:], bias_sb[:], op=OP.add)
                sq = sm.tile([128, CL], f32, tag="sq")
                nc.vector.tensor_tensor(sq[:], ss[:], ss[:], op=OP.mult)
                n2 = sm.tile([128, C], f32, tag="n2")
                nc.vector.tensor_reduce(
                    n2[:], sq[:].rearrange("p (c l) -> p c l", l=L),
                    axis=AX.X, op=OP.add)
                rt = sm.tile([128, C], f32, tag="rt")
                nc.scalar.sqrt(rt[:], n2[:])
                d1 = sm.tile([128, C], f32, tag="d1")
                nc.vector.tensor_scalar_add(d1[:], n2[:], 1.0)
                d2 = sm.tile([128, C], f32, tag="d2")
                nc.vector.tensor_scalar_add(d2[:], rt[:], EPS)
                nc.vector.tensor_tensor(d1[:], d1[:], d2[:], op=OP.mult)
                rec = sm.tile([128, C], f32, tag="rec")
                nc.vector.reciprocal(rec[:], d1[:])
                nc.vector.tensor_tensor(rec[:], rec[:], n2[:], op=OP.mult)
                nc.vector.tensor_tensor(
                    vout[:].rearrange("p (c l) -> p c l", l=L),
                    ss[:].rearrange("p (c l) -> p c l", l=L),
                    rec[:].unsqueeze(2).to_broadcast([128, C, L]),
                    op=OP.mult)

            # ---------- iteration 0: c uniform -> s0 = sum(u)/C ----------
            with tc.tile_pool(name="ps_acc", bufs=1, space="PSUM") as ps_acc:
                pacc = ps_acc.tile([128, 4, 512], f32, tag="pacc")
                for t in range(NT):
                    quad_mms(pacc, t, acc=True, first=(t == 0),
                             last=(t == NT - 1))
                s0 = sm.tile([128, CL], f32, tag="s0")
                nc.vector.tensor_reduce(
                    s0[:],
                    pacc[:, :, 0:320].rearrange("p q (n k) -> p k q n", n=2),
                    axis=AX.XY, op=OP.add)
            s0f = all_reduce(s0, "r0")
            squash(s0f, 1.0 / C, v_sb, "r0")

            # ---------- iterations 1 and 2 ----------
            for it in (1, 2):
                with tc.tile_pool(name=f"ps_{it}z", bufs=2,
                                  space="PSUM") as psz:
                    for t in range(NT):
                        pq = psz.tile([128, 4, 512], f32, tag="pq")
                        quad_mms(pq, t)
                        u_ap = pq[:, :, 0:320].rearrange(
                            "p q (n c l) -> p q n c l", n=2, c=C, l=L)
                        zb = wk.tile([128, 1280], f32, tag="zb")
                        nc.vector.tensor_tensor(
                            zb[:].rearrange("p (q n c l) -> p q n c l",
                                            q=4, n=2, c=C, l=L),
                            u_ap,
                            v_sb[:].rearrange("p (c l) -> p c l", l=L)
                            .unsqueeze(1).unsqueeze(1)
                            .to_broadcast([128, 4, 2, C, L]),
                            op=OP.mult)
                        if it == 1:
                            nc.vector.tensor_reduce(
                                bstate[:, 80 * t:80 * (t + 1)],
                                zb[:].rearrange("p (a l) -> p a l", l=L),
                                axis=AX.X, op=OP.add)
                        else:
                            bt = wk.tile([128, 80], f32, tag="bt")
                            nc.vector.tensor_reduce(
                                bt[:],
                                zb[:].rearrange("p (a l) -> p a l", l=L),
                                axis=AX.X, op=OP.add)
                            nc.vector.tensor_tensor(
                                bstate[:, 80 * t:80 * (t + 1)],
                                bstate[:, 80 * t:80 * (t + 1)],
                                bt[:], op=OP.add)

                # softmax over c (groups of 10 in bstate cols)
                nc.scalar.activation(cstate[:], bstate[:], AF.Exp)
                den = sm.tile([128, NL], f32, tag="den")
                nc.vector.tensor_reduce(
                    den[:], cstate[:].rearrange("p (n c) -> p n c", c=C),
                    axis=AX.X, op=OP.add)
                rec = sm.tile([128, NL], f32, tag="recn")
                nc.vector.reciprocal(rec[:], den[:])
                nc.vector.tensor_tensor(
                    cstate[:].rearrange("p (n c) -> p n c", c=C),
                    cstate[:].rearrange("p (n c) -> p n c", c=C),
                    rec[:].unsqueeze(2).to_broadcast([128, NL, C]),
                    op=OP.mult)

                # s = sum_n c*u  (second einsum sweep)
                nc.any.memzero(s_acc)
                with tc.tile_pool(name=f"ps_{it}y", bufs=2,
                                  space="PSUM") as psy:
                    for t in range(NT):
                        pq = psy.tile([128, 4, 512], f32, tag="pqy")
                        quad_mms(pq, t)
                        yb = wk.tile([128, 1280], f32, tag="zb")
                        nc.vector.tensor_tensor(
                            yb[:].rearrange("p (c l q n) -> p c l q n",
                                            c=C, l=L, q=4, n=2),
                            pq[:, :, 0:320].rearrange(
                                "p q (n c l) -> p c l q n", n=2, c=C, l=L),
                            cstate[:, 80 * t:80 * (t + 1)].rearrange(
                                "p (q n c) -> p c q n", q=4, n=2)
                            .unsqueeze(2).to_broadcast([128, C, L, 4, 2]),
                            op=OP.mult)
                        st = wk.tile([128, CL], f32, tag="st")
                        nc.vector.tensor_reduce(
                            st[:],
                            yb[:].rearrange("p (k a) -> p k a", a=8),
                            axis=AX.X, op=OP.add)
                        nc.vector.tensor_tensor(
                            s_acc[:], s_acc[:], st[:], op=OP.add)
                sf = all_reduce(s_acc, f"r{it}")
                squash(sf, 1.0, v_sb, f"r{it}")

            nc.sync.dma_start(out[:], v_sb[:])

    nc.compile()
    return nc


def _build_nc_v2(sim=False, dbg=False):
    """No-u design: routing contractions on TensorE in n-on-partitions layout.

    Per iteration: bupd = x*(W@v) via Wv-matmuls + delta-matmul partition-group
    reduces; softmax in [(n),(c,b)] layout; s via y=c*x as matmul stationary
    against W. PSUM accumulation replaces all big DVE reductions.
    """
    import concourse.bass as bass  # noqa: F401
    import concourse.mybir as mybir
    import concourse.tile as tile
    import concourse.bacc as bacc
    from concourse.masks import make_identity

    f32 = mybir.dt.float32
    bf16 = mybir.dt.bfloat16
    AX = mybir.AxisListType
    OP = mybir.AluOpType
    AF = mybir.ActivationFunctionType

    nc = bacc.Bacc("TRN2", target_bir_lowering=False, debug=False,
                   num_devices=1 if sim else NCORES)

    xt_d = nc.dram_tensor("xt", [128, 5, 8, 128], bf16, kind="ExternalInput")
    xnj_d = nc.dram_tensor("xnj", [128, 5, 8, 128], bf16, kind="ExternalInput")
    wt4_d = nc.dram_tensor("wt4", [128, 5, 8, 160], bf16, kind="ExternalInput")
    wt6_d = nc.dram_tensor("wt6", [128, 3, 5, 8, 128], bf16, kind="ExternalInput")
    delta_d = nc.dram_tensor("delta", [128, 32], bf16, kind="ExternalInput")
    bias = nc.dram_tensor("bias", [128, CL], f32, kind="ExternalInput")
    out = nc.dram_tensor("out", [128, CL], f32, kind="ExternalOutput")
    if dbg:
        s0_dbg = nc.dram_tensor("s0_dbg", [128, CL], f32, kind="ExternalOutput")
        v0_dbg = nc.dram_tensor("v0_dbg", [128, CL], f32, kind="ExternalOutput")
        b1_dbg = nc.dram_tensor("b1_dbg", [128, 5, 10, 128], f32, kind="ExternalOutput")
        c1_dbg = nc.dram_tensor("c1_dbg", [128, 5, 10, 128], f32, kind="ExternalOutput")
        s1_dbg = nc.dram_tensor("s1_dbg", [128, CL], f32, kind="ExternalOutput")
    RG = [list(range(NCORES))]

    with tile.TileContext(nc) as tc:
        with (
            tc.tile_pool(name="big", bufs=1) as big,
            tc.tile_pool(name="dram", bufs=1, space="DRAM") as dram,
            tc.tile_pool(name="wk", bufs=3) as wk,
            tc.tile_pool(name="sm", bufs=1) as sm,
        ):
            xt = big.tile([128, 5, 8, 128], bf16, tag="xt")
            xnj = big.tile([128, 5, 8, 128], bf16, tag="xnj")
            wt4 = big.tile([128, 5, 8, 160], bf16, tag="wt4")
            wt6 = big.tile([128, 3, 5, 8, 128], bf16, tag="wt6")
            delta_sb = big.tile([128, 32], bf16, tag="delta")
            bias_sb = big.tile([128, CL], f32, tag="bias_sb")
            ident = big.tile([128, 128], f32, tag="ident")
            bstateT = big.tile([128, 5, 10, 128], f32, tag="bstateT")
            cstateT = big.tile([128, 5, 10, 128], bf16, tag="cstateT")
            den_t = big.tile([128, 5, 128], f32, tag="den_t")
            rec_t = big.tile([128, 5, 128], f32, tag="rec_t")
            vT = big.tile([128, 3, 128], bf16, tag="vT")
            v_sb = big.tile([128, CL], f32, tag="v_sb")

            make_identity(nc, ident[:])
            for G in range(5):
                nc.sync.dma_start(xt[:, G], xt_d[:, G])
                nc.sync.dma_start(xnj[:, G], xnj_d[:, G])
                nc.sync.dma_start(wt4[:, G], wt4_d[:, G])
            for cyc in range(3):
                nc.sync.dma_start(wt6[:, cyc], wt6_d[:, cyc])
            nc.sync.dma_start(delta_sb[:], delta_d[:])
            nc.sync.dma_start(bias_sb[:], bias[:])

            def all_reduce(src_sb, tag):
                cin = dram.tile([128, CL], f32, tag=f"cc_in_{tag}")
                cout = dram.tile([128, CL], f32, tag=f"cc_out_{tag}")
                nc.sync.dma_start(cin[:], src_sb[:])
                if not sim:
                    nc.gpsimd.collective_compute(
                        "AllReduce", OP.add, replica_groups=RG,
                        ins=[cin[:].opt()], outs=[cout[:].opt()])
                dst = sm.tile([128, CL], f32, tag="cc_sb")
                nc.sync.dma_start(dst[:], cin[:] if sim else cout[:])
                return dst

            def squash(s_red, scale, vout):
                ss = sm.tile([128, CL], f32, tag="ss")
                nc.scalar.activation(ss[:], s_red[:], AF.Copy, scale=float(scale))
                nc.vector.tensor_tensor(ss[:], ss[:], bias_sb[:], op=OP.add)
                sq = sm.tile([128, CL], f32, tag="sq")
                nc.vector.tensor_tensor(sq[:], ss[:], ss[:], op=OP.mult)
                n2 = sm.tile([128, C], f32, tag="n2")
                nc.vector.tensor_reduce(
                    n2[:], sq[:].rearrange("p (c l) -> p c l", l=L),
                    axis=AX.X, op=OP.add)
                rt = sm.tile([128, C], f32, tag="rt")
                nc.scalar.sqrt(rt[:], n2[:])
                d1 = sm.tile([128, C], f32, tag="d1")
                nc.vector.tensor_scalar_add(d1[:], n2[:], 1.0)
                d2 = sm.tile([128, C], f32, tag="d2")
                nc.vector.tensor_scalar_add(d2[:], rt[:], EPS)
                nc.vector.tensor_tensor(d1[:], d1[:], d2[:], op=OP.mult)
                rec = sm.tile([128, C], f32, tag="rec")
                nc.vector.reciprocal(rec[:], d1[:])
                nc.vector.tensor_tensor(rec[:], rec[:], n2[:], op=OP.mult)
                nc.vector.tensor_tensor(
                    vout[:].rearrange("p (c l) -> p c l", l=L),
                    ss[:].rearrange("p (c l) -> p c l", l=L),
                    rec[:].unsqueeze(2).to_broadcast([128, C, L]),
                    op=OP.mult)

            def build_vT(vsrc):
                with tc.tile_pool(name="pst", bufs=2, space="PSUM") as pst:
                    for c in range(10):
                        slot, cyc = c % 4, c // 4
                        tp = pst.tile([16, 128], f32, tag="tp")
                        nc.tensor.transpose(
                            tp[:], vsrc[:, 16 * c:16 * c + 16], ident[:])
                        nc.scalar.copy(
                            vT[32 * slot:32 * slot + 16, cyc, :], tp[:])

            def s_from_psum(sp):
                s_sb = sm.tile([128, CL], f32, tag="s_sb")
                nc.vector.tensor_copy(
                    s_sb[:].rearrange("p (c l) -> p c l", l=L), sp[:])
                return s_sb

            # ---------- iteration 0: s0 = (1/C) sum_{n,j} x W ----------
            with tc.tile_pool(name="ps0", bufs=1, space="PSUM") as ps0:
                sp = ps0.tile([128, 10, 16], f32, tag="sp0")
                for c in range(10):
                    for G in range(5):
                        for j in range(8):
                            nc.tensor.matmul(
                                sp[:, c, :], xt[:, G, j, :],
                                wt4[:, G, j, 16 * c:16 * c + 16],
                                start=(G == 0 and j == 0),
                                stop=(G == 4 and j == 7),
                                skip_group_check=True)
                s0 = s_from_psum(sp)
            if dbg:
                nc.sync.dma_start(s0_dbg[:], s0[:])
            s0f = all_reduce(s0, "r0")
            squash(s0f, 1.0 / C, v_sb)
            if dbg:
                nc.sync.dma_start(v0_dbg[:], v_sb[:])
            build_vT(v_sb)

            # ---------- iterations 1 and 2 ----------
            for it in (1, 2):
                with (tc.tile_pool(name=f"wv{it}", bufs=2, space="PSUM") as wvp,
                      tc.tile_pool(name=f"bu{it}", bufs=1, space="PSUM") as bup):
                    for G in range(5):
                        bq = bup.tile([128, 10, 128], f32, tag="bq")
                        for c in range(10):
                            slot, cyc = c % 4, c // 4
                            wq = wvp.tile([128, 8, 128], f32, tag="wq")
                            for h in range(8):
                                nc.tensor.matmul(
                                    wq[:, h, :],
                                    wt6[32 * slot:32 * slot + 16, cyc, G, h, :],
                                    vT[32 * slot:32 * slot + 16, cyc, :],
                                    start=True, stop=True,
                                    tile_position=(96, 0) if slot == 3 else None)
                            wvs = wk.tile([128, 8, 128], bf16, tag="wvs")
                            nc.scalar.copy(wvs[:], wq[:])
                            zt = wk.tile([128, 8, 128], bf16, tag="zt")
                            nc.vector.tensor_tensor(
                                zt[:], wvs[:], xnj[:, G], op=OP.mult)
                            for s4 in range(4):
                                for ii, h in enumerate((s4, s4 + 4)):
                                    nc.tensor.matmul(
                                        bq[32 * s4:32 * s4 + 32, c, :],
                                        delta_sb[:], zt[:, h, :],
                                        start=(ii == 0), stop=(ii == 1),
                                        tile_position=(0, 32 * s4),
                                        skip_group_check=True)
                        if it == 1:
                            nc.scalar.copy(bstateT[:, G], bq[:])
                        else:
                            nc.vector.tensor_tensor(
                                bstateT[:, G], bstateT[:, G], bq[:], op=OP.add)

                if dbg and it == 1:
                    nc.sync.dma_start(b1_dbg[:], bstateT[:])
                # softmax over c in [(n), (c, b)] layout
                nc.scalar.activation(cstateT[:], bstateT[:], AF.Exp)
                for G in range(5):
                    nc.vector.tensor_reduce(
                        den_t[:, G, :],
                        cstateT[:, G].rearrange("p c b -> p b c"),
                        axis=AX.X, op=OP.add)
                nc.vector.reciprocal(rec_t[:], den_t[:])
                for G in range(5):
                    nc.vector.tensor_tensor(
                        cstateT[:, G], cstateT[:, G],
                        rec_t[:, G, :].unsqueeze(1).to_broadcast([128, 10, 128]),
                        op=OP.mult)

                if dbg and it == 1:
                    cdbg = big.tile([128, 5, 10, 128], f32, tag="cdbg")
                    nc.vector.tensor_copy(cdbg[:], cstateT[:])
                    nc.sync.dma_start(c1_dbg[:], cdbg[:])
                # s = sum_{n,j} (c*x) W  via y-stationary matmuls
                # NOTE: accumulation chains must be sequential per PSUM
                # region - interleaved chains sharing a bank corrupt results.
                with tc.tile_pool(name=f"sps{it}", bufs=1, space="PSUM") as sps:
                    sp = sps.tile([128, 10, 16], f32, tag="spi")
                    for c in range(10):
                        for G in range(5):
                            y = wk.tile([128, 8, 128], bf16, tag="y")
                            nc.vector.tensor_tensor(
                                y[:],
                                cstateT[:, G, c, :].unsqueeze(1)
                                .to_broadcast([128, 8, 128]),
                                xt[:, G], op=OP.mult)
                            for j in range(8):
                                nc.tensor.matmul(
                                    sp[:, c, :], y[:, j, :],
                                    wt4[:, G, j, 16 * c:16 * c + 16],
                                    start=(G == 0 and j == 0),
                                    stop=(G == 4 and j == 7),
                                    skip_group_check=True)
                    s_it = s_from_psum(sp)
                if dbg and it == 1:
                    nc.sync.dma_start(s1_dbg[:], s_it[:])
                sf = all_reduce(s_it, f"r{it}")
                squash(sf, 1.0, v_sb)
                if it == 1:
                    build_vT(v_sb)

            nc.sync.dma_start(out[:], v_sb[:])

    nc.compile()
    return nc


def _build_nc_v3(sim=False):
    """v3: G-pipelined iterations. Same inputs/host-prep as v2.

    Differences vs v2:
    - iter0 uses 40 matmuls with 160-wide rhs (was 400 x 16-wide).
    - iterations fuse bupd -> softmax -> s into ONE loop over G so the
      Tile scheduler can overlap TensorE/Scalar/Vector across G-blocks
      (per-G chains are independent; s accumulates in per-c PSUM chains
      ordered G0..G4).
    - softmax normalize is folded into x: xhat = x * (1/den), y = e * xhat
      (skips the cstate normalize pass; everything stays bf16 2x).
    - half the wq->SBUF copies go to VectorE to unblock ScalarE.
    """
    import concourse.bass as bass  # noqa: F401
    import concourse.mybir as mybir
    import concourse.tile as tile
    import concourse.bacc as bacc
    from concourse.masks import make_identity

    f32 = mybir.dt.float32
    bf16 = mybir.dt.bfloat16
    AX = mybir.AxisListType
    OP = mybir.AluOpType
    AF = mybir.ActivationFunctionType

    nc = bacc.Bacc("TRN2", target_bir_lowering=False, debug=False,
                   num_devices=1 if sim else NCORES)

    xt_d = nc.dram_tensor("xt", [128, 5, 8, 128], bf16, kind="ExternalInput")
    xnj_d = nc.dram_tensor("xnj", [128, 5, 8, 128], bf16, kind="ExternalInput")
    wt4_d = nc.dram_tensor("wt4", [128, 5, 8, 160], bf16, kind="ExternalInput")
    wt6_d = nc.dram_tensor("wt6", [128, 3, 5, 8, 128], bf16, kind="ExternalInput")
    delta_d = nc.dram_tensor("delta", [128, 32], bf16, kind="ExternalInput")
    bias = nc.dram_tensor("bias", [128, CL], f32, kind="ExternalInput")
    out = nc.dram_tensor("out", [128, CL], f32, kind="ExternalOutput")
    RG = [list(range(NCORES))]

    with tile.TileContext(nc) as tc:
        with (
            tc.tile_pool(name="big", bufs=1) as big,
            tc.tile_pool(name="dram", bufs=1, space="DRAM") as dram,
            tc.tile_pool(name="wk", bufs=4) as wk,
            tc.tile_pool(name="sm", bufs=1) as sm,
        ):
            xt = big.tile([128, 5, 8, 128], bf16, tag="xt")
            xnj = big.tile([128, 5, 8, 128], bf16, tag="xnj")
            wt4 = big.tile([128, 5, 8, 160], bf16, tag="wt4")
            wt6 = big.tile([128, 3, 5, 8, 128], bf16, tag="wt6")
            delta_sb = big.tile([128, 32], bf16, tag="delta")
            bias_sb = big.tile([128, CL], f32, tag="bias_sb")
            ident = big.tile([128, 128], f32, tag="ident")
            bstateT = big.tile([128, 5, 10, 128], f32, tag="bstateT")
            cstateT = big.tile([128, 5, 10, 128], bf16, tag="cstateT")
            vT = big.tile([128, 3, 128], bf16, tag="vT")
            v_sb = big.tile([128, CL], f32, tag="v_sb")

            make_identity(nc, ident[:])
            # Warm-up barrier: a tiny AllReduce issued first so the 8 cores
            # rendezvous while input DMA + iter0 matmuls run.  Without it the
            # first real AllReduce absorbs all the launch skew (~25us).
            warm_in = dram.tile([128, 4], f32, tag="warm_in")
            warm_out = dram.tile([128, 4], f32, tag="warm_out")
            warm_sb = sm.tile([128, 4], f32, tag="warm_sb")
            nc.any.memzero(warm_sb)
            nc.sync.dma_start(warm_in[:], warm_sb[:])
            if not sim:
                nc.gpsimd.collective_compute(
                    "AllReduce", OP.add, replica_groups=RG,
                    ins=[warm_in[:].opt()], outs=[warm_out[:].opt()])
            # iter0 needs only xt+wt4: load those first so matmuls start
            # early; xnj/wt6/delta (iteration-1 operands) stream in behind,
            # overlapped with iter0 compute + the first collective.
            for G in range(5):
                nc.sync.dma_start(xt[:, G], xt_d[:, G])
                nc.sync.dma_start(wt4[:, G], wt4_d[:, G])
            nc.sync.dma_start(bias_sb[:], bias[:])
            for G in range(5):
                nc.sync.dma_start(xnj[:, G], xnj_d[:, G])
            nc.sync.dma_start(delta_sb[:], delta_d[:])
            for cyc in range(3):
                nc.sync.dma_start(wt6[:, cyc], wt6_d[:, cyc])

            def all_reduce(src_sb, tag):
                cin = dram.tile([128, CL], f32, tag=f"cc_in_{tag}")
                cout = dram.tile([128, CL], f32, tag=f"cc_out_{tag}")
                nc.sync.dma_start(cin[:], src_sb[:])
                if not sim:
                    nc.gpsimd.collective_compute(
                        "AllReduce", OP.add, replica_groups=RG,
                        ins=[cin[:].opt()], outs=[cout[:].opt()])
                dst = sm.tile([128, CL], f32, tag="cc_sb")
                nc.sync.dma_start(dst[:], cin[:] if sim else cout[:])
                return dst

            def squash(s_red, scale, vout):
                ss = sm.tile([128, CL], f32, tag="ss")
                nc.scalar.activation(ss[:], s_red[:], AF.Copy, scale=float(scale))
                nc.vector.tensor_tensor(ss[:], ss[:], bias_sb[:], op=OP.add)
                sq = sm.tile([128, CL], f32, tag="sq")
                nc.vector.tensor_tensor(sq[:], ss[:], ss[:], op=OP.mult)
                n2 = sm.tile([128, C], f32, tag="n2")
                nc.vector.tensor_reduce(
                    n2[:], sq[:].rearrange("p (c l) -> p c l", l=L),
                    axis=AX.X, op=OP.add)
                rt = sm.tile([128, C], f32, tag="rt")
                nc.scalar.sqrt(rt[:], n2[:])
                d1 = sm.tile([128, C], f32, tag="d1")
                nc.vector.tensor_scalar_add(d1[:], n2[:], 1.0)
                d2 = sm.tile([128, C], f32, tag="d2")
                nc.vector.tensor_scalar_add(d2[:], rt[:], EPS)
                nc.vector.tensor_tensor(d1[:], d1[:], d2[:], op=OP.mult)
                rec = sm.tile([128, C], f32, tag="rec")
                nc.vector.reciprocal(rec[:], d1[:])
                nc.vector.tensor_tensor(rec[:], rec[:], n2[:], op=OP.mult)
                nc.vector.tensor_tensor(
                    vout[:].rearrange("p (c l) -> p c l", l=L),
                    ss[:].rearrange("p (c l) -> p c l", l=L),
                    rec[:].unsqueeze(2).to_broadcast([128, C, L]),
                    op=OP.mult)

            def build_vT(vsrc):
                with tc.tile_pool(name="pst", bufs=2, space="PSUM") as pst:
                    for c in range(10):
                        slot, cyc = c % 4, c // 4
                        tp = pst.tile([16, 128], f32, tag="tp")
                        nc.tensor.transpose(
                            tp[:], vsrc[:, 16 * c:16 * c + 16], ident[:])
                        nc.scalar.copy(
                            vT[32 * slot:32 * slot + 16, cyc, :], tp[:])

            # ---------- iteration 0: s0 = (1/C) sum_{n,j} x W ----------
            with tc.tile_pool(name="ps0", bufs=1, space="PSUM") as ps0:
                sp = ps0.tile([128, 10, 16], f32, tag="sp0")
                for G in range(5):
                    for j in range(8):
                        nc.tensor.matmul(
                            sp[:].rearrange("p c l -> p (c l)"),
                            xt[:, G, j, :], wt4[:, G, j, :],
                            start=(G == 0 and j == 0),
                            stop=(G == 4 and j == 7))
                s0 = sm.tile([128, CL], f32, tag="s_sb0")
                nc.vector.tensor_copy(
                    s0[:].rearrange("p (c l) -> p c l", l=L), sp[:])
            s0f = all_reduce(s0, "r0")
            squash(s0f, 1.0 / C, v_sb)
            build_vT(v_sb)

            # ---------- iterations 1 and 2 (G-pipelined) ----------
            for it in (1, 2):
                with (
                    tc.tile_pool(name=f"wv{it}", bufs=2, space="PSUM") as wvp,
                    tc.tile_pool(name=f"bu{it}", bufs=1, space="PSUM") as bup,
                    tc.tile_pool(name=f"sp{it}", bufs=1, space="PSUM") as spp,
                ):
                    s_acc = sm.tile([128, CL], f32, tag=f"sacc{it}")
                    for G in range(5):
                        # --- b-update for this G ---
                        bq = bup.tile([128, 10, 128], f32, tag="bq")
                        for c in range(10):
                            slot, cyc = c % 4, c // 4
                            wq = wvp.tile([128, 8, 128], f32, tag="wq")
                            for h in range(8):
                                nc.tensor.matmul(
                                    wq[:, h, :],
                                    wt6[32 * slot:32 * slot + 16, cyc, G, h, :],
                                    vT[32 * slot:32 * slot + 16, cyc, :],
                                    start=True, stop=True,
                                    tile_position=(96, 0) if slot == 3
                                    else None)
                            wvs = wk.tile([128, 8, 128], bf16, tag="wvs")
                            nc.scalar.copy(wvs[:], wq[:])
                            zt = wk.tile([128, 8, 128], bf16, tag="zt")
                            nc.vector.tensor_tensor(
                                zt[:], wvs[:], xnj[:, G], op=OP.mult)
                            for s4 in range(4):
                                for ii, h in enumerate((s4, s4 + 4)):
                                    nc.tensor.matmul(
                                        bq[32 * s4:32 * s4 + 32, c, :],
                                        delta_sb[:], zt[:, h, :],
                                        start=(ii == 0), stop=(ii == 1),
                                        tile_position=(0, 32 * s4),
                                        skip_group_check=True)
                        if it == 1:
                            nc.scalar.copy(bstateT[:, G], bq[:])
                        else:
                            nc.vector.tensor_tensor(
                                bstateT[:, G], bstateT[:, G], bq[:], op=OP.add)

                        # --- softmax for this G (normalize folded into x) ---
                        nc.scalar.activation(
                            cstateT[:, G], bstateT[:, G], AF.Exp)
                        den = sm.tile([128, 128], f32, tag=f"den{it}_{G}")
                        nc.vector.tensor_reduce(
                            den[:],
                            cstateT[:, G].rearrange("p c b -> p b c"),
                            axis=AX.X, op=OP.add)
                        recb = sm.tile([128, 128], bf16, tag=f"recb{it}_{G}")
                        with nc.allow_low_precision(
                                reason="softmax 1/den in bf16; rel tol 2e-2"):
                            nc.vector.reciprocal(recb[:], den[:])
                        xh = wk.tile([128, 8, 128], bf16, tag="xh")
                        nc.vector.tensor_tensor(
                            xh[:], xt[:, G],
                            recb[:].unsqueeze(1).to_broadcast([128, 8, 128]),
                            op=OP.mult)

                        # --- s contribution for this G (per-c chains close
                        # within G; cross-G accumulation in SBUF to avoid
                        # interleaved chains sharing a PSUM bank) ---
                        sp = spp.tile([128, 10, 16], f32, tag="spi")
                        for c in range(10):
                            y = wk.tile([128, 8, 128], bf16, tag="y")
                            nc.vector.tensor_tensor(
                                y[:],
                                cstateT[:, G, c, :].unsqueeze(1)
                                .to_broadcast([128, 8, 128]),
                                xh[:], op=OP.mult)
                            for j in range(8):
                                nc.tensor.matmul(
                                    sp[:, c, :], y[:, j, :],
                                    wt4[:, G, j, 16 * c:16 * c + 16],
                                    start=(j == 0), stop=(j == 7),
                                    skip_group_check=True)
                        if G == 0:
                            nc.vector.tensor_copy(
                                s_acc[:].rearrange("p (c l) -> p c l", l=L),
                                sp[:])
                        else:
                            nc.vector.tensor_tensor(
                                s_acc[:].rearrange("p (c l) -> p c l", l=L),
                                s_acc[:].rearrange("p (c l) -> p c l", l=L),
                                sp[:], op=OP.add)
                    s_it = s_acc
                sf = all_reduce(s_it, f"r{it}")
                squash(sf, 1.0, v_sb)
                if it == 1:
                    build_vT(v_sb)

            nc.sync.dma_start(out[:], v_sb[:])

    nc.compile()
    return nc


def _prep_core_v2(x_shard, W_shard):
    """Host layouts for v2. x_shard [128,576,8] f32, W_shard [576,8,160]."""
    import ml_dtypes
    bf = ml_dtypes.bfloat16
    xp = np.zeros((128, 640, 8), np.float32)
    xp[:, :NL] = x_shard
    Wp = np.zeros((640, 8, 10, 16), np.float32)
    Wp[:NL] = W_shard.reshape(NL, 8, 10, 16)

    # xt[nn, G, j, b] = xp[b, 128G+nn, j]
    xt = xp.reshape(128, 5, 128, 8).transpose(2, 1, 3, 0).copy()
    # xnj[4*ns+jj, G, 4*jh+s, b] = xp[b, 128G+32s+ns, 4jh+jj]
    t = xp.reshape(128, 5, 4, 32, 2, 4)           # [b, G, s, ns, jh, jj]
    xnj = (t.transpose(3, 5, 1, 4, 2, 0)          # [ns, jj, G, jh, s, b]
           .reshape(128, 5, 8, 128))
    # wt4[nn, G, j, (c l)] = Wp[128G+nn, j, c, l]
    wt4 = (Wp.reshape(5, 128, 8, 160).transpose(1, 0, 2, 3)).copy()
    # wt6[32*(c%4)+l, c//4, G, 4*jh+s, 4*ns+jj] = Wp[128G+32s+ns, 4jh+jj, c, l]
    wt6 = np.zeros((128, 3, 5, 8, 128), np.float32)
    t6 = Wp.reshape(5, 4, 32, 2, 4, 10, 16)       # [G, s, ns, jh, jj, c, l]
    for c in range(10):
        blk = t6[:, :, :, :, :, c, :]             # [G, s, ns, jh, jj, l]
        arr = (blk.transpose(5, 0, 3, 1, 2, 4)    # [l, G, jh, s, ns, jj]
               .reshape(16, 5, 8, 128))
        wt6[32 * (c % 4):32 * (c % 4) + 16, c // 4] = arr
    delta = np.zeros((128, 32), np.float32)
    for ns in range(32):
        delta[4 * ns:4 * ns + 4, ns] = 1.0
    return (xt.astype(bf), xnj.astype(bf), wt4.astype(bf), wt6.astype(bf),
            delta.astype(bf))


def _prep_core(x_shard, W_shard):
    """x_shard [B,576,8] f32, W_shard [576,8,160] f32 -> (xs, wm) operands."""
    xs = np.zeros((128, NT, 128), np.float32)
    wmv = np.zeros((128, NT, 320), np.float32)
    xr = x_shard.transpose(1, 2, 0).reshape(NT, 4, 2, J, B)
    wr = W_shard.reshape(NT, 4, 2, J, CL)
    for vv in range(4):
        for ns in range(2):
            rows = slice(32 * vv + 8 * ns, 32 * vv + 8 * ns + 8)
            xs[rows] = xr[:, vv, ns].transpose(1, 0, 2)
            wmv[rows, :, CL * ns:CL * (ns + 1)] = wr[:, vv, ns].transpose(1, 0, 2)
    return xs.reshape(128, NT * 128), wmv.reshape(128, NT * 320)


def prepare(inputs):
    """Build (nc, in_maps) for the current kernel version."""
    x = np.ascontiguousarray(inputs["inputs"], dtype=np.float32)
    W = np.ascontiguousarray(inputs["W"], dtype=np.float32)
    biases = np.ascontiguousarray(inputs["biases"], dtype=np.float32)
    x = x.reshape(B, NTOT, J)
    bias_rep = np.broadcast_to(biases.reshape(1, CL), (128, CL)).copy()

    import os
    ver = os.environ.get("KERNEL_V", "3")
    in_maps = []
    for i in range(NCORES):
        sl = slice(i * NL, (i + 1) * NL)
        if ver in ("2", "3"):
            xt, xnj, wt4, wt6, delta = _prep_core_v2(x[:, sl], W[sl])
            in_maps.append({"xt": xt, "xnj": xnj, "wt4": wt4, "wt6": wt6,
                            "delta": delta, "bias": bias_rep})
        else:
            xs, wmv = _prep_core(x[:, sl], W[sl])
            in_maps.append({"xs": xs, "wm": wmv, "bias": bias_rep})

    key = f"nc{ver}"
    if key not in _NC_CACHE:
        builder = {"1": _build_nc, "2": _build_nc_v2, "3": _build_nc_v3}[ver]
        _NC_CACHE[key] = builder()
    return _NC_CACHE[key], in_maps


def kernel(**inputs):
    from concourse.bass_utils import run_bass_kernel_spmd

    nc, in_maps = prepare(inputs)
    res = run_bass_kernel_spmd(nc, in_maps, core_ids=list(range(NCORES)))
    return res.results[0]["out"].reshape(B, C, L).astype(np.float32)

